# revision 36
# baseline (speedup 1.0000x reference)
"""Trainium2 kernel for the OpticalFront dense net.

Reference computation:
    xr = Re(idft2(tmask * dft2(x)))          # centered 2D FFT front
    h = relu(xr.flat @ w1.T + b1)
    out = log_softmax(h @ w4.T + b4)

The optical front is a fixed real-linear operator A on each flattened
28x28 image (xr_flat = x_flat @ A.T), so it folds into the first FC
layer on the host: w1_eff = w1 @ A.  The device then runs a pure GEMM
pipeline, data-parallel over 8 NeuronCores (4096 images per core):

    H.T[hid,  b] = sum_k W1T[k, hid].T @ XT[k, b]     (bf16, fp32 acc)
    L.T[10,   b] = sum_k W4T[k, 10].T  @ H.T[k, b]
    out.T[10, b] = (L.T + b4) - ln(ones.T @ exp(L.T + b4))

PE-array packing (the contraction K=785 and output M=800 are not
multiples of 128, and fc2/softmax have tiny M):
  * fc1's last k-tile has 17 live rows; pairs of m-tiles run their k6
    matmuls concurrently in disjoint 32-row strips (tile_position row
    packing) against row-replicated copies of x[768:785]/w1[:,768:785].
  * fc1's last m-tile (32 cols), fc2 (M=10), and both softmax matmuls
    (M=1 / M=10) for CP=4 batch chunks run concurrently in disjoint
    32-column strips (col packing); w4's k6 block, b4 and the softmax
    ones operands are partition-strip replicated so contraction rows
    line up.
Warm-up matmuls on scratch data run during the initial DMA so the PE's
HAM clock gate reaches 2.4 GHz before real work arrives; weights load
on the scalar HWDGE queue and x on the sync HWDGE queue in first-use
order.
"""

import numpy as np
import ml_dtypes

import concourse.bass as bass
import concourse.bacc as bacc
import concourse.mybir as mybir
import concourse.tile as tile
from concourse.bass_utils import run_bass_kernel_spmd

BF16 = mybir.dt.bfloat16
F32 = mybir.dt.float32
AF = mybir.ActivationFunctionType

B, H, W = 32768, 28, 28
PIX = H * W            # 784
HID = 800
NCLS = 10
NCORES = 8
BPC = B // NCORES      # 4096 images per core
NB = 512               # batch chunk = one PSUM bank of fp32
NCH = BPC // NB        # 8 chunks per core
K6 = 17                # leftover pixel rows 768:784 + bias/ones row
CP = 4                 # batch chunks packed into PE column strips
NG = NCH // CP         # chunk groups per core
DUMMIES = 7            # HAM warm-up matmuls during the startup DMA
SMAX_P = 32 * (CP - 1) + NCLS   # 106: partition span covering all strips

_built = None  # (nc, input_names) cache — BIR build is pure host work


def _build_device_program():
    nc = bacc.Bacc(
        "TRN2", target_bir_lowering=False, debug=False, num_devices=NCORES
    )
    xt_d = nc.dram_tensor("xt", [768, BPC], BF16, kind="ExternalInput")
    xt6_d = nc.dram_tensor("xt6", [49, BPC], BF16, kind="ExternalInput")
    w1t_d = nc.dram_tensor("w1t", [768, HID], BF16, kind="ExternalInput")
    w1t6_d = nc.dram_tensor("w1t6", [49, HID], BF16, kind="ExternalInput")
    w4t_d = nc.dram_tensor("w4t", [7 * 128, NCLS], BF16, kind="ExternalInput")
    b4_d = nc.dram_tensor("b4", [128, 1], F32, kind="ExternalInput")
    out_d = nc.dram_tensor("outT", [NCLS, BPC], F32, kind="ExternalOutput")

    # Pre-load the one ACT-function table that contains every function we
    # use (relu/exp/ln/identity) so the LUT never reloads mid-kernel.
    from concourse.hw_specs import get_activation_tables
    needed = {AF.Relu, AF.Exp, AF.Ln, AF.Identity, AF.Copy}
    table_id = None
    for i, (name, funcs) in enumerate(get_activation_tables(nc.m.arch).items()):
        if needed <= funcs:
            table_id = i
            break

    with tile.TileContext(nc) as tc:
        if table_id is not None:
            nc.scalar.add_instruction(
                mybir.InstLoadActFuncSet(
                    name=nc.get_next_instruction_name(),
                    act_func_set_id=table_id,
                    ins=[],
                    outs=[],
                )
            )
        with (
            tc.tile_pool(name="static", bufs=1) as wpool,
            tc.tile_pool(name="hmid", bufs=1) as hpool,
            tc.tile_pool(name="smax", bufs=2) as spool,
            tc.tile_pool(name="psum_h", bufs=1, space="PSUM") as psum_h,
            tc.tile_pool(name="psum_s", bufs=1, space="PSUM") as psum_s,
        ):
            # Scratch data for HAM warm-up matmuls (results never read).
            dumw = wpool.tile([128, 128], BF16)
            dumx = wpool.tile([128, NB], BF16)
            nc.vector.memset(dumw[:, :], 0.0)
            nc.vector.memset(dumx[:, :], 0.0)
            ones_sb = wpool.tile([128, NCLS], BF16)
            nc.vector.memset(ones_sb[:, :], 1.0)

            # Weights on the scalar HWDGE queue, first-use first.
            w1_view = w1t_d.ap().rearrange("(k p) m -> p k m", p=128)
            w4_view = w4t_d.ap().rearrange("(k p) m -> p k m", p=128)
            w1_sb = wpool.tile([128, 6, HID], BF16)
            w1_6sb = wpool.tile([49, HID], BF16)
            w4_sb = wpool.tile([128, 7, NCLS], BF16)
            b4_sb = wpool.tile([128, 1], F32)
            nc.scalar.dma_start(w1_sb[:, 0:1, :], w1_view[:, 0:1, :])
            nc.scalar.dma_start(w1_sb[:, 1:2, :], w1_view[:, 1:2, :])
            nc.scalar.dma_start(w1_6sb[:, :], w1t6_d[:, :])
            nc.scalar.dma_start(b4_sb[:, :], b4_d[:, :])

            # All of x stays resident in SBUF; stream it on the sync HWDGE
            # queue in consumption order (first chunk in small pieces).
            xt_view = xt_d.ap().rearrange("(k p) n -> p k n", p=128)
            xt_sb = wpool.tile([128, 6, BPC], BF16)
            xt6_sb = wpool.tile([49, BPC], BF16)
            nc.sync.dma_start(xt_sb[:, :, 0:512], xt_view[:, :, 0:512])
            nc.sync.dma_start(xt6_sb[:, 0:512], xt6_d[:, 0:512])
            nc.sync.dma_start(w1_sb[:, 2:3, :], w1_view[:, 2:3, :])
            nc.sync.dma_start(w1_sb[:, 3:4, :], w1_view[:, 3:4, :])
            nc.sync.dma_start(w1_sb[:, 4:5, :], w1_view[:, 4:5, :])
            nc.sync.dma_start(w1_sb[:, 5:6, :], w1_view[:, 5:6, :])
            nc.sync.dma_start(w4_sb[:, :, :], w4_view)
            XSPLIT = [(512, 1024), (1024, 2048), (2048, 3072), (3072, 4096)]
            for (c0, c1) in XSPLIT:
                nc.sync.dma_start(xt_sb[:, :, c0:c1], xt_view[:, :, c0:c1])
                nc.sync.dma_start(xt6_sb[:, c0:c1], xt6_d[:, c0:c1])

            # Warm-up matmuls: run during the DMA wait so the PE's HAM
            # clock gate un-throttles before the first real matmul.
            dum_ps = psum_h.tile([128, NB], F32, tag="ph", bufs=4)
            for _ in range(DUMMIES):
                nc.tensor.matmul(dum_ps[:, :], dumw[:, :], dumx[:, :])

            MS = [(m * 128, min(128, HID - m * 128)) for m in range(7)]

            # Deferred per-group softmax tail (PE part) — emitted inside
            # the *next* group's compute so the PE never stalls on
            # ScalarE's exp/ln.
            pending = None

            def softmax_pe(state):
                pl, exp_sb, logit_sb, g = state
                ps = psum_s.tile([128, NB], F32, tag="ps")
                for c in range(CP):
                    s = 32 * c
                    nc.tensor.matmul(
                        ps[s:s + 1, :], ones_sb[s:s + NCLS, 0:1],
                        exp_sb[s:s + NCLS, :], tile_position=(s, s),
                    )
                # One wide ln covers all CP strip rows (plus garbage lanes
                # in between, which nothing reads).
                lse_sb = spool.tile([128, NB], BF16, tag="lse")
                nc.scalar.activation(
                    lse_sb[0:32 * (CP - 1) + 1, :], ps[0:32 * (CP - 1) + 1, :],
                    AF.Ln,
                )
                pb = psum_s.tile([128, NB], F32, tag="pb")
                for c in range(CP):
                    s = 32 * c
                    nc.tensor.matmul(
                        pb[s:s + NCLS, :], ones_sb[s:s + 1, 0:NCLS],
                        lse_sb[s:s + 1, :], tile_position=(s, s),
                    )
                out_sb = spool.tile([128, NB], F32, tag="outc")
                nc.vector.tensor_sub(
                    out_sb[0:SMAX_P, :], logit_sb[0:SMAX_P, :], pb[0:SMAX_P, :],
                )
                nb0 = g * CP
                for c in range(CP):
                    s = 32 * c
                    nb = nb0 + c
                    # Alternate queues: each dma_start costs ~750ns of
                    # engine issue time, so two queues halve the tail.
                    eng = nc.sync if c % 2 == 0 else nc.scalar
                    eng.dma_start(
                        out_d[:, nb * NB:(nb + 1) * NB],
                        out_sb[s:s + NCLS, :],
                    )

            for g in range(NG):
                hts = []  # hts[c][m] for m<6
                for c in range(CP):
                    nbs = slice((g * CP + c) * NB, (g * CP + c + 1) * NB)
                    htc = []
                    for pair in range(3):
                        ma, mb = 2 * pair, 2 * pair + 1
                        pha = psum_h.tile([128, NB], F32, tag="ph", bufs=4)
                        phb = psum_h.tile([128, NB], F32, tag="ph", bufs=4)
                        first = (g == 0 and c == 0 and pair == 0)
                        for k in range(6):
                            nc.tensor.matmul(
                                pha[:, :], w1_sb[:, k, ma * 128:ma * 128 + 128],
                                xt_sb[:, k, nbs], start=(k == 0), stop=False,
                            )
                            if first and k < 5:
                                # No-dependency filler matmuls: they run
                                # while the next w1 k-tile's DMA lands, so
                                # the PE stays busy and HAM un-throttles.
                                for _ in range(2 if k in (1, 2, 3) else 1):
                                    nc.tensor.matmul(
                                        dum_ps[:, :], dumw[:, :], dumx[:, :]
                                    )
                        for k in range(6):
                            nc.tensor.matmul(
                                phb[:, :], w1_sb[:, k, mb * 128:mb * 128 + 128],
                                xt_sb[:, k, nbs], start=(k == 0), stop=False,
                            )
                        # k6 leftovers: two concurrent row-strip matmuls.
                        nc.tensor.matmul(
                            pha[:, :], w1_6sb[0:K6, ma * 128:ma * 128 + 128],
                            xt6_sb[0:K6, nbs], start=False, stop=True,
                            tile_position=(0, 0),
                        )
                        nc.tensor.matmul(
                            phb[:, :], w1_6sb[32:32 + K6, mb * 128:mb * 128 + 128],
                            xt6_sb[32:32 + K6, nbs], start=False, stop=True,
                            tile_position=(32, 0),
                        )
                        hta = hpool.tile([128, NB], BF16, tag="ht", bufs=52)
                        htb = hpool.tile([128, NB], BF16, tag="ht", bufs=52)
                        # Split relus across ScalarE and VectorE so neither
                        # queue backs up and stalls fc2's ht waits.
                        nc.scalar.activation(hta[:, :], pha[:, :], AF.Relu)
                        nc.vector.tensor_scalar_max(htb[:, :], phb[:, :], 0.0)
                        htc += [hta, htb]
                    hts.append(htc)

                # m6 (hid 768:800, 32 cols): CP chunks in column strips.
                # Shares the "pl" ring: ph6 is drained by relu before the
                # group's fc2 allocates pl, so two slots cover both.
                ph6 = psum_h.tile([128, NB], F32, tag="pl", bufs=2)
                for k in range(6):
                    for c in range(CP):
                        s = 32 * c
                        nbs = slice((g * CP + c) * NB, (g * CP + c + 1) * NB)
                        nc.tensor.matmul(
                            ph6[s:s + 32, :], w1_sb[:, k, 768:800],
                            xt_sb[:, k, nbs], start=(k == 0), stop=False,
                            tile_position=(0, s),
                        )
                for c in range(CP):
                    s = 32 * c
                    nbs = slice((g * CP + c) * NB, (g * CP + c + 1) * NB)
                    nc.tensor.matmul(
                        ph6[s:s + 32, :], w1_6sb[0:K6, 768:800],
                        xt6_sb[0:K6, nbs], start=False, stop=True,
                        tile_position=(0, s),
                    )
                # All 128 partitions of ph6 are live strips: one wide relu,
                # on VectorE so it is not queued behind ScalarE's backlog
                # (fc2's k6 round waits on it).
                ht6 = hpool.tile([128, NB], BF16, tag="ht6", bufs=2)
                nc.vector.tensor_scalar_max(ht6[:, :], ph6[:, :], 0.0)

                # Previous group's softmax PE work goes here: its exp/ln
                # inputs are long since ready, so no PE stall.
                if pending is not None:
                    softmax_pe(pending)
                    pending = None

                # fc2: CP chunks in column strips, contraction over hid.
                pl = psum_h.tile([128, NB], F32, tag="pl", bufs=2)
                for k in range(6):
                    for c in range(CP):
                        s = 32 * c
                        nc.tensor.matmul(
                            pl[s:s + NCLS, :], w4_sb[:, k, :], hts[c][k][:, :],
                            start=(k == 0), stop=False, tile_position=(0, s),
                        )
                for c in range(CP):
                    s = 32 * c
                    nc.tensor.matmul(
                        pl[s:s + NCLS, :], w4_sb[s:s + 32, 6, :],
                        ht6[s:s + 32, :], start=False, stop=True,
                        tile_position=(s, s),
                    )
                # One wide exp over all strips; garbage lanes never read.
                exp_sb = spool.tile([128, NB], BF16, tag="exp")
                nc.scalar.activation(
                    exp_sb[0:SMAX_P, :], pl[0:SMAX_P, :], AF.Exp,
                    bias=b4_sb[0:SMAX_P, :],
                )
                # logit = pl + b4 now, so the pl PSUM bank is released
                # before the next group's fc2 needs it.
                logit_sb = spool.tile([128, NB], F32, tag="logit")
                nc.vector.tensor_scalar_add(
                    logit_sb[0:SMAX_P, :], pl[0:SMAX_P, :], b4_sb[0:SMAX_P, :],
                )
                pending = (pl, exp_sb, logit_sb, g)
            softmax_pe(pending)

    nc.finalize()
    return nc


def _optical_operator(tmask_re, tmask_im):
    """A such that xr_flat = A @ x_flat for the masked centered FFT front."""
    tmask = tmask_re.astype(np.complex64) + 1j * tmask_im.astype(np.complex64)
    tmask = tmask.reshape(H, W)
    ax = (-2, -1)
    eye = np.eye(PIX, dtype=np.complex64).reshape(PIX, H, W)
    f = np.fft.fftshift(np.fft.fft2(np.fft.ifftshift(eye, axes=ax), axes=ax), axes=ax)
    f *= tmask[None, :, :]
    xr = np.fft.fftshift(np.fft.ifft2(np.fft.ifftshift(f, axes=ax), axes=ax), axes=ax)
    return np.real(xr).reshape(PIX, PIX).T.astype(np.float64)


def kernel(x, tmask_re, tmask_im, w1, b1, w4, b4):
    global _built
    x = np.asarray(x)
    w1 = np.asarray(w1, dtype=np.float32)
    b1 = np.asarray(b1, dtype=np.float32)
    w4 = np.asarray(w4, dtype=np.float32)
    b4 = np.asarray(b4, dtype=np.float32)
    tre = np.asarray(tmask_re, dtype=np.float32)
    tim = np.asarray(tmask_im, dtype=np.float32)

    # Fold the optical front into w1.  Identity mask -> A == I exactly.
    if np.all(tre == 1.0) and np.all(tim == 0.0):
        w1e = w1.astype(np.float64)
    else:
        w1e = w1.astype(np.float64) @ _optical_operator(tre, tim)

    bf16 = ml_dtypes.bfloat16
    w1ef = w1e.T.astype(np.float32)           # [784, 800]
    w1t = np.ascontiguousarray(w1ef[:768, :]).astype(bf16)
    w1t6 = np.zeros((49, HID), dtype=bf16)
    w1t6[:16, :] = w1ef[768:784, :]
    w1t6[16, :] = b1                          # bias row, pairs with ones row
    w1t6[32:49, :] = w1t6[:17, :]             # replica for row strip 1

    w4t = np.zeros((7 * 128, NCLS), dtype=bf16)
    w4t[:768, :] = w4.T[:768, :]
    for c in range(CP):                       # k6 block per column strip
        w4t[768 + 32 * c:768 + 32 * c + 32, :] = w4.T[768:800, :]
    b4c = np.zeros((128, 1), dtype=np.float32)
    for c in range(CP):
        b4c[32 * c:32 * c + NCLS, 0] = b4

    xf = x.reshape(B, PIX).astype(bf16).T     # [784, B]
    xt = np.ascontiguousarray(xf[:768, :])
    xt6 = np.zeros((49, B), dtype=bf16)
    xt6[:16, :] = xf[768:784, :]
    xt6[16, :] = 1.0                          # ones row for the b1 fold
    xt6[32:49, :] = xt6[:17, :]               # replica for row strip 1

    if _built is None:
        _built = _build_device_program()
    nc = _built

    in_maps = [
        {
            "xt": np.ascontiguousarray(xt[:, c * BPC:(c + 1) * BPC]),
            "xt6": np.ascontiguousarray(xt6[:, c * BPC:(c + 1) * BPC]),
            "w1t": w1t,
            "w1t6": w1t6,
            "w4t": w4t,
            "b4": b4c,
        }
        for c in range(NCORES)
    ]
    res = run_bass_kernel_spmd(nc, in_maps, core_ids=list(range(NCORES)))

    out = np.empty((B, NCLS), dtype=np.float32)
    for c in range(NCORES):
        out[c * BPC:(c + 1) * BPC, :] = res.results[c]["outT"].T
    return out
